# revision 41
# baseline (speedup 1.0000x reference)
"""Self-contained TRN2 Bass kernel for nn_Attention (B=4, N=2048, D=1024, H=16).

Sharding: 8 NeuronCores, core c = (batch b = c//2, head-half = c%2).
Each core computes causal attention for its batch and 8 of 16 heads plus the
row-parallel half of the output projection; the host sums the two half-partials
per batch.

Per-core pipeline (all on-device, Tile-scheduled):
  q-proj bf16; k-proj fp8e4m3 DoubleRow (64x weight scale folded into the
    host-side q scale); v-proj bf16 -> V [token, feat] (+ones col)
  S^T chunk [128 k-tok, 512 q-tok] = kT-part @ qT   (bf16, f32 psum)
  causal mask added on the diagonal blocks via one paired PE matmul
  exp on ScalarE -> P bf16
  O^T + softmax denominator via P^T @ [V | 1]
  1/den: both heads' den rows folded to a [128, 8] tile by DMA, DVE
    reciprocal there, DRAM bounce for the partition broadcast
  partial out [NT, E] = OT.T @ woutT  (bf16 matmuls, f32 psum); the last
    stores split across engines to keep the tail off a single DMA queue
"""

import os
import sys
import types
from contextlib import ExitStack
from dataclasses import dataclass

for _p in ('/opt/trn_rl_repo', '/root/.axon_site/_ro/trn_rl_repo'):
    if os.path.isdir(_p) and _p not in sys.path:
        sys.path.append(_p)

import numpy as np
import ml_dtypes

import concourse.bass as bass
import concourse.mybir as mybir
import concourse.tile as tile
from concourse import bacc

F32 = mybir.dt.float32
F32R = mybir.dt.float32r
BF16 = mybir.dt.bfloat16
F8 = mybir.dt.float8e4


# ---------------------------------------------------------------- harness fixes
def _install_ntff_hook():
    """Register the axon NTFF profile hook that trn_boot skips when the
    container's antenv stub lacks axon_hooks (needed only for trace=True)."""
    if 'antenv.axon_hooks' in sys.modules:
        return
    try:
        import antenv
        mod = types.ModuleType('antenv.axon_hooks')
        _hook = [None]
        mod.set_axon_ntff_profile_hook = lambda h: _hook.__setitem__(0, h)
        mod.get_axon_ntff_profile_hook = lambda: _hook[0]
        sys.modules['antenv.axon_hooks'] = mod
        antenv.axon_hooks = mod
        from trn_agent_boot.trn_boot import _ntff_profile_via_ctypes
        so = '/opt/axon/libaxon_pjrt.so'
        if os.path.exists(so):
            hook = _ntff_profile_via_ctypes(so)
            if hook is not None:
                mod.set_axon_ntff_profile_hook(hook)
    except Exception:
        pass


def _patch_tile_drain():
    """walrus TPB_CTRL encodes <=2 sync waits; Tile's tail drain can carry
    more. Split extras onto single-wait nops (sequentially equivalent)."""
    import concourse.tile as tile_mod
    if getattr(tile_mod.TileContext, '_drain_patched', False):
        return
    from concourse.tile import ScopedClock

    def _drain_and_barrier(self, tick_clock, wait_clock):
        nc = self.nc
        drain_inst = nc.sync.drain()
        wait_clock.add_sem_waits(
            drain_inst.ins, ScopedClock({None: tick_clock.global_clock}))
        si = drain_inst.ins.sync_info
        if si is not None and si.on_wait and len(si.on_wait) > 1:
            waits = list(si.on_wait)
            drain_inst.ins.sync_info = mybir.SyncInfo(
                on_wait=waits[:1], on_update=list(si.on_update or []))
            for w in waits[1:]:
                nop = nc.sync.nop(nofuse=True)
                nop.ins.sync_info = mybir.SyncInfo(on_wait=[w], on_update=[])
        nc.all_engine_barrier()
        popped = nc._tile_sem_poison_stack.pop()
        assert popped is self._sem_poison
        nc.clear_and_free_semaphores(list(self.sems.allocated().values()))
        nc.all_engine_barrier()

    tile_mod.TileContext._drain_and_barrier = _drain_and_barrier
    tile_mod.TileContext._drain_patched = True


# ---------------------------------------------------------------- kernel build
@dataclass(frozen=True)
class Cfg:
    NT: int = 2048   # tokens
    D: int = 1024    # model dim
    HH: int = 8      # heads per core
    DH: int = 64     # head dim
    E: int = 1024    # output features
    QC: int = 512    # q-chunk (free dim of score tiles)
    KC: int = 128    # k-tile (partition dim of score tiles)
    MASK_NEG: float = -1e30
    PT_BUFS: int = 8
    PSS_BUFS: int = 2
    PSO_BUFS: int = 2
    OT_BUFS: int = 8
    DEN_BUFS: int = 6
    XT_BUFS: int = 2
    PSA_BUFS: int = 4

    @property
    def DC(self): return self.D // 128
    @property
    def NKT(self): return self.NT // self.KC
    @property
    def NQC(self): return self.NT // self.QC
    @property
    def NPAIR(self): return self.HH // 2
    @property
    def QF(self): return self.HH * self.DH
    @property
    def VW(self): return self.DH + 1


def build(cfg: Cfg) -> bass.Bass:
    _patch_tile_drain()
    nc = bacc.Bacc('TRN2', target_bir_lowering=False)
    c = cfg
    assert c.QC % c.KC == 0 and c.NT % c.QC == 0 and c.D % 128 == 0
    assert c.DH * 2 == c.KC
    JPT = c.QC // c.KC

    xT = nc.declare_dram_parameter("xT", [128, c.DC, c.NT], BF16, isOutput=False)
    wqk = nc.declare_dram_parameter("wqk", [128, c.DC, c.QF], BF16, isOutput=False)
    wk8 = nc.declare_dram_parameter("wk8", [128, c.DC // 2, 2, c.QF], F8,
                                    isOutput=False)
    wv = nc.declare_dram_parameter("wv", [128, c.DC, c.QF], BF16, isOutput=False)
    wout = nc.declare_dram_parameter("wout", [128, c.NPAIR, c.E], BF16, isOutput=False)
    mtri = nc.declare_dram_parameter("mtri", [128, 2, c.KC], BF16, isOutput=False)
    iden = nc.declare_dram_parameter("iden", [128, 128], BF16, isOutput=False)
    out = nc.declare_dram_parameter("out", [c.NT, c.E], F32, isOutput=True)

    with tile.TileContext(nc) as tc, ExitStack() as ctx:
        const = ctx.enter_context(tc.tile_pool(name="const", bufs=1))
        persist = ctx.enter_context(tc.tile_pool(name="persist", bufs=1))

        ones64 = const.tile([c.VW, c.DH], F32)
        nc.vector.memset(ones64[c.DH:c.VW, :], 1.0)
        mtri_sb = const.tile([128, 2, c.KC], BF16)
        iden_sb = const.tile([128, 128], BF16)
        wv_sb = const.tile([128, c.DC, c.QF], BF16)
        wqk_sb = const.tile([128, c.DC, c.QF], BF16)
        wk8_sb = const.tile([128, c.DC // 2, 2, c.QF], F8)
        wout_sb = const.tile([128, c.NPAIR, c.E], BF16)

        qk_sb = [persist.tile([128, c.NT], BF16, tag=f"qk{e}", name=f"qk{e}")
                 for e in range(2 * c.NPAIR)]
        V_sb = persist.tile([128, c.NKT, c.HH, c.VW], BF16, tag="V", name="V_sb")
        nc.vector.memset(V_sb[:, :, :, c.DH], 1.0)
        OT_sb = [persist.tile([128, c.NT], BF16, tag=f"ot{p}", name=f"ot{p}")
                 for p in range(c.NPAIR)]

        xt2 = persist.tile([128, c.DC, c.NT], BF16, tag="xt", name="xt2")
        x8 = persist.tile([128, c.DC, c.NT], F8, tag="x8", name="x8")
        # Input DMA schedule.  Fine-grained transfers spread across DMA
        # queues (a single big DMA rides one engine at ~1/16 of the HBM
        # bandwidth).  Critical path on sync: x chunk 0 + wv gate Phase A,
        # then wqk's pair-0 columns gate the q/k prologue.  Everything
        # else is issued from the idle GpSimd engine's software DGE.
        ch = c.QC
        for dc in range(c.DC):
            # x chunk 0 + wv split across the sync and scalar DGEs so the
            # triggers issue in parallel; dc 0/1 transfers are halved so
            # Phase A's first matmuls are not gated on a full 128KB
            # single-engine transfer
            eng = nc.sync if dc % 2 == 0 else nc.scalar
            if dc < 2:
                h = ch // 2
                eng.dma_start(out=xt2[:, dc, 0:h], in_=xT[:, dc, 0:h])
                eng.dma_start(out=xt2[:, dc, h:ch], in_=xT[:, dc, h:ch])
                hf = c.QF // 2
                eng.dma_start(out=wv_sb[:, dc, 0:hf], in_=wv[:, dc, 0:hf])
                eng.dma_start(out=wv_sb[:, dc, hf:], in_=wv[:, dc, hf:])
            else:
                eng.dma_start(out=xt2[:, dc, 0:ch], in_=xT[:, dc, 0:ch])
                eng.dma_start(out=wv_sb[:, dc, :], in_=wv[:, dc, :])
        nc.sync.dma_start(out=mtri_sb[:], in_=mtri[:])
        nc.scalar.dma_start(out=iden_sb[:], in_=iden[:])
        for dc in range(c.DC):
            # pair-0 q feature columns + the fp8 k weights (k projection
            # runs in fp8 DoubleRow off a casted copy of x)
            eng = nc.sync if dc % 2 == 0 else nc.scalar
            eng.dma_start(
                out=wqk_sb[:, dc, 0:128], in_=wqk[:, dc, 0:128])
            if dc < c.DC // 2:
                eng.dma_start(out=wk8_sb[:, dc], in_=wk8[:, dc])
        # x chunk 0 casted to fp8 (gpsimd DGE casts in flight) — needed by
        # the k prologue first
        for d2 in range(0, c.DC, 2):
            nc.gpsimd.dma_start(
                out=x8[:, d2:d2 + 2, 0:ch], in_=xT[:, d2:d2 + 2, 0:ch])
        for dc in range(c.DC):
            # x chunk 1 (needed by pair-0 fillers early)
            eng = nc.sync if dc % 2 == 0 else nc.scalar
            eng.dma_start(out=xt2[:, dc, ch:2 * ch], in_=xT[:, dc, ch:2 * ch])
        for dc in range(c.DC):
            # remaining pairs' q columns (pair-1 chunks are emitted as
            # fillers from pair 0's first iteration already)
            nc.gpsimd.dma_start(
                out=wqk_sb[:, dc, 128:c.QF], in_=wqk[:, dc, 128:c.QF])
        # interleave the casted-x and bf16-x chunk streams by need time
        for cs in range(ch, c.NT, ch):
            for d2 in range(0, c.DC, 2):
                nc.gpsimd.dma_start(
                    out=x8[:, d2:d2 + 2, cs:cs + ch],
                    in_=xT[:, d2:d2 + 2, cs:cs + ch])
            if cs + ch < c.NT:
                for d2 in range(0, c.DC, 2):
                    nc.gpsimd.dma_start(
                        out=xt2[:, d2:d2 + 2, cs + ch:cs + 2 * ch],
                        in_=xT[:, d2:d2 + 2, cs + ch:cs + 2 * ch])
        for pr in range(c.NPAIR):
            nc.gpsimd.dma_start(
                out=wout_sb[:, pr, :], in_=wout[:, pr, :])

        # ---------------- Phase A: v-projection (first q-chunk only;
        # the rest is emitted as filler work inside pair 0) ----------------
        NTT0 = c.QC // 128
        with tc.tile_pool(name="ps_a", bufs=c.PSA_BUFS, space="PSUM") as ps_a:
            # dc-outer order so each arriving x/wv chunk is consumed
            # immediately across all four psum tiles
            psvs = [ps_a.tile([128, c.QF], F32, tag="ps", name=f"psv{nt}")
                    for nt in range(min(NTT0, c.NKT))]
            for dc in range(c.DC):
                for nt, psv in enumerate(psvs):
                    nc.tensor.matmul(
                        psv[:],
                        lhsT=xt2[:, dc, nt * 128:(nt + 1) * 128],
                        rhs=wv_sb[:, dc, :],
                        start=(dc == 0), stop=(dc == c.DC - 1),
                    )
            for nt, psv in enumerate(psvs):
                nc.vector.tensor_copy(
                    out=V_sb[:, nt, :, 0:c.DH],
                    in_=psv[:].rearrange("p (h f) -> p h f", h=c.HH),
                )

        # ---------------- Phase B: attention ----------------
        with (
            tc.tile_pool(name="pt", bufs=c.PT_BUFS) as pt_pool,
            tc.tile_pool(name="otst", bufs=c.OT_BUFS) as ot_pool,
            tc.tile_pool(name="den", bufs=c.DEN_BUFS) as den_pool,
            tc.tile_pool(name="dend", bufs=4, space="DRAM") as dend_pool,
            tc.tile_pool(name="ps_s", bufs=c.PSS_BUFS, space="PSUM") as ps_s,
            tc.tile_pool(name="ps_o", bufs=c.PSO_BUFS, space="PSUM") as ps_o,
            tc.tile_pool(name="ps_f", bufs=2, space="PSUM") as ps_f,
            tc.tile_pool(name="osbB", bufs=3) as outB_pool,
        ):
            def evac_stage1(psO):
                ocps = []
                for h2 in range(2):
                    # evacuate psum to SBUF immediately to free the bank
                    # (GPSIMD cannot read PSUM, so both stay on DVE)
                    ocp = ot_pool.tile([c.VW, c.QC], F32, tag="ocp",
                                       name="ocp", bufs=6)
                    nc.vector.tensor_copy(out=ocp[:], in_=psO[h2][:])
                    ocps.append(ocp)
                return ocps

            def evac_stage2(p, t, ocps):
                qsl_full = slice(t * c.QC, (t + 1) * c.QC)
                # Fold both heads' denominator rows [1, QC] into a [128, *]
                # square via DMA so the DVE reciprocal runs across all 128
                # lanes (~130ns instead of ~3.3us on one lane), then bounce
                # through DRAM for the partition broadcast.  Both hops use
                # identical element orderings, so positions are preserved.
                QW = c.QC // 128
                den_sq = den_pool.tile([128, 2, QW], F32, tag="dsq",
                                       name="den_sq", bufs=3)
                for h2 in range(2):
                    nc.sync.dma_start(
                        out=den_sq[:, h2, :], in_=ocps[h2][c.DH:c.VW, :])
                rec_sq = den_pool.tile([128, 2, QW], F32, tag="rsq",
                                       name="rec_sq", bufs=3)
                nc.vector.reciprocal(out=rec_sq[:], in_=den_sq[:])
                rec_d = dend_pool.tile([2, c.QC], F32, tag="dend",
                                       name="rec_d")
                for h2 in range(2):
                    nc.sync.dma_start(
                        out=rec_d[h2], in_=rec_sq[:, h2, :])
                for h2 in range(2):
                    ocp = ocps[h2]
                    divB = den_pool.tile([c.DH, c.QC], F32, tag="div",
                                         name="divB")
                    nc.sync.dma_start(
                        out=divB[:], in_=rec_d[h2].partition_broadcast(c.DH))
                    if h2 == 0:
                        nc.vector.tensor_tensor(
                            out=OT_sb[p][0:c.DH, qsl_full], in0=ocp[0:c.DH, :],
                            in1=divB[:], op=mybir.AluOpType.mult,
                        )
                    else:
                        ot_st = ot_pool.tile([c.DH, c.QC], BF16, tag="ot",
                                             name="ot_st")
                        nc.gpsimd.tensor_tensor(
                            out=ot_st[:], in0=ocp[0:c.DH, :], in1=divB[:],
                            op=mybir.AluOpType.mult,
                        )
                        nc.sync.dma_start(
                            out=OT_sb[p][c.DH:2 * c.DH, qsl_full],
                            in_=ot_st[:])

            def emit_proj_chunk(nt, ec, split=False):
                esl = slice(ec * c.QC, (ec + 1) * c.QC)
                psP = ps_f.tile([128, c.QC], F32, tag="f", name="psP")
                for pr in range(c.NPAIR):
                    nc.tensor.matmul(
                        psP[:],
                        lhsT=OT_sb[pr][:, nt * 128:(nt + 1) * 128],
                        rhs=wout_sb[:, pr, esl],
                        start=(pr == 0), stop=(pr == c.NPAIR - 1),
                    )
                o_sb = outB_pool.tile([128, c.QC], F32, tag="ob", name="o_sb")
                nc.vector.tensor_copy(out=o_sb[:], in_=psP[:])
                if not split:
                    nc.sync.dma_start(
                        out=out[nt * 128:(nt + 1) * 128, esl], in_=o_sb[:])
                else:
                    # the run's last stores: 4-way partition split across
                    # three trigger engines so the 256KB transfer does not
                    # ride a single 22GB/s DMA engine on the critical tail
                    for i, eng in enumerate(
                            (nc.sync, nc.scalar, nc.gpsimd, nc.sync)):
                        ps = slice(i * 32, (i + 1) * 32)
                        eng.dma_start(
                            out=out[nt * 128 + i * 32:
                                    nt * 128 + (i + 1) * 32, esl],
                            in_=o_sb[ps, :])

            def emit_qk_chunk(pp, ci):
                is_k = ci // c.NQC
                e = (pp, c.NPAIR + pp)[is_k]
                ncc = ci % c.NQC
                nsl = slice(ncc * c.QC, (ncc + 1) * c.QC)
                psqk = ps_f.tile([128, c.QC], F32, tag="f", name="psqk")
                if is_k:
                    # fp8 DoubleRow: two D-chunks per pass, 0.5 cycles/row
                    for dcp in range(c.DC // 2):
                        nc.tensor.matmul(
                            psqk[:],
                            lhsT=wk8_sb[:, dcp, :, pp * 128:(pp + 1) * 128],
                            rhs=x8[:, 2 * dcp:2 * dcp + 2, nsl],
                            start=(dcp == 0), stop=(dcp == c.DC // 2 - 1),
                            perf_mode=mybir.MatmulPerfMode.DoubleRow,
                        )
                else:
                    for dc in range(c.DC):
                        nc.tensor.matmul(
                            psqk[:],
                            lhsT=wqk_sb[:, dc, pp * 128:(pp + 1) * 128],
                            rhs=xt2[:, dc, nsl],
                            start=(dc == 0), stop=(dc == c.DC - 1),
                        )
                nc.vector.tensor_copy(out=qk_sb[e][:, nsl], in_=psqk[:])

            def emit_v_chunk(nt):
                psv = ps_f.tile([128, c.QF], F32, tag="f", name="psvf")
                for dc in range(c.DC):
                    nc.tensor.matmul(
                        psv[:],
                        lhsT=xt2[:, dc, nt * 128:(nt + 1) * 128],
                        rhs=wv_sb[:, dc, :],
                        start=(dc == 0), stop=(dc == c.DC - 1),
                    )
                nc.vector.tensor_copy(
                    out=V_sb[:, nt, :, 0:c.DH],
                    in_=psv[:].rearrange("p (h f) -> p h f", h=c.HH),
                )

            pending2 = None
            # prologue: only the first q-chunk's columns of q and k
            emit_qk_chunk(0, 0)
            emit_qk_chunk(0, c.NQC)
            for p in range(c.NPAIR):
                q_t, k_t = qk_sb[p], qk_sb[c.NPAIR + p]
                for t in range(c.NQC):
                    njt = JPT * t + JPT
                    psO = [ps_o.tile([c.VW, c.QC], F32, tag="o", name=f"psO{_h}")
                           for _h in range(2)]
                    def emit_pv(items):
                        for (h2_, pt_, lo_, j_) in items:
                            nc.tensor.matmul(
                                psO[h2_][:, lo_:],
                                lhsT=V_sb[:, j_, 2 * p + h2_, :],
                                rhs=pt_[:, h2_, lo_:],
                                start=(j_ == 0), stop=(j_ == njt - 1),
                            )

                    pipe = []
                    for j in range(njt):
                        off = j * c.KC - t * c.QC
                        band = off >= 0
                        lo = max(off, 0)
                        jsl = slice(j * c.KC, (j + 1) * c.KC)
                        qsl = slice(t * c.QC + lo, (t + 1) * c.QC)
                        # both heads' scores into one 2-bank psum tile
                        psS = ps_s.tile([128, 2, c.QC], F32, tag="s", name="psS")
                        for h2 in range(2):
                            hsl = slice(h2 * c.DH, (h2 + 1) * c.DH)
                            nc.tensor.matmul(
                                psS[:, h2, lo:], lhsT=k_t[hsl, jsl],
                                rhs=q_t[hsl, qsl], start=True, stop=(not band),
                            )
                        if band:
                            # causal mask add on PE for both heads at once:
                            # psum += I.T @ [mtri | mtri]
                            nc.tensor.matmul(
                                psS[:, :, off:off + c.KC], lhsT=iden_sb[:],
                                rhs=mtri_sb[:], start=False, stop=True,
                                skip_group_check=True,
                            )
                        pt_t = pt_pool.tile([128, 2, c.QC], BF16, tag="pt",
                                            name="pt_t")
                        nc.scalar.activation(
                            out=pt_t[:, :, lo:],
                            in_=psS[:, :, lo:],
                            func=mybir.ActivationFunctionType.Exp,
                        )
                        pipe.append([(0, pt_t, lo, j), (1, pt_t, lo, j)])
                        if len(pipe) > 3:
                            emit_pv(pipe.pop(0))
                    # interleave filler work (next pair's qk projection,
                    # or output-projection chunks during the last pair) with
                    # the PV pipe flush so the PE has matmuls to run while
                    # ScalarE finishes the tail exps
                    NTT = c.QC // 128
                    if p == 0:
                        fillers = []
                        if t + 1 < c.NQC:
                            # next q-chunk's V rows and q/k columns
                            fillers += [
                                lambda nt_=nt_: emit_v_chunk(nt_)
                                for nt_ in range((t + 1) * NTT,
                                                 min((t + 2) * NTT, c.NKT))]
                            fillers += [
                                lambda ci=ci: emit_qk_chunk(0, ci)
                                for ci in (t + 1, c.NQC + t + 1)]
                        fillers += [lambda i=i: emit_qk_chunk(1, 2 * t + i)
                                    for i in range(2)]
                    elif p + 1 < c.NPAIR:
                        fillers = [lambda i=i: emit_qk_chunk(p + 1, 2 * t + i)
                                   for i in range(2)]
                    elif t >= 2:
                        # two-chunk lag keeps the filler projections clear
                        # of the still-in-flight evac chain of chunk t-1
                        tp = t - 2
                        fillers = [
                            lambda nt_=nt_, ec_=ec_: emit_proj_chunk(nt_, ec_)
                            for nt_ in range(tp * NTT, (tp + 1) * NTT)
                            for ec_ in range(c.E // c.QC)]
                    else:
                        fillers = []
                    while fillers or pipe:
                        if fillers:
                            fillers.pop(0)()
                        if pipe:
                            emit_pv(pipe.pop(0))
                    ocps = evac_stage1(psO)
                    if pending2 is not None:
                        evac_stage2(*pending2)
                        pending2 = None
                    if p == c.NPAIR - 1:
                        # last pair: evacuate immediately so the output
                        # projection can chase with only one chunk of lag
                        evac_stage2(p, t, ocps)
                    else:
                        pending2 = (p, t, ocps)

            if pending2 is not None:
                evac_stage2(*pending2)

            # final q-chunks' output projection, still inside the same pool
            # scope (no drain barrier before it); chunk NQC-2 is ready
            # immediately and overlaps the last chunk's evac chain
            NTTf = c.QC // 128
            for nt in range((c.NQC - 2) * NTTf, c.NT // 128):
                for ec in range(c.E // c.QC):
                    emit_proj_chunk(nt, ec, split=(nt >= c.NT // 128 - 2))

    nc.compile()
    return nc


# ---------------------------------------------------------------- host side
def make_core_inputs(xb, w_qkv, w_out, mask, cfg, half):
    c = cfg
    D = c.D
    K8_SCALE = 64.0
    scale = 1.0 / np.sqrt(c.DH) / K8_SCALE  # k carries a 64x fp8 scale
    heads = range(half * c.HH, (half + 1) * c.HH)
    q_rows = np.concatenate(
        [w_qkv[h * c.DH:(h + 1) * c.DH, :] for h in heads]) * scale
    k_rows = np.concatenate(
        [w_qkv[D + h * c.DH:D + (h + 1) * c.DH, :] for h in heads])
    v_rows = np.concatenate(
        [w_qkv[2 * D + h * c.DH:2 * D + (h + 1) * c.DH, :] for h in heads])
    wqk_t = np.ascontiguousarray(
        q_rows.T.reshape(c.DC, 128, c.QF).transpose(1, 0, 2)).astype(
            ml_dtypes.bfloat16)
    # [p, dcp, o, f] = 64*wk[f, (2*dcp+o)*128 + p] in fp8e4m3
    wk8_t = np.ascontiguousarray(
        (k_rows.T * K8_SCALE).reshape(c.DC // 2, 2, 128, c.QF)
        .transpose(2, 0, 1, 3)).astype(ml_dtypes.float8_e4m3fn)
    wv_t = np.ascontiguousarray(
        v_rows.T.reshape(c.DC, 128, c.QF).transpose(1, 0, 2)).astype(
            ml_dtypes.bfloat16)
    wo = w_out[:, half * c.QF:(half + 1) * c.QF].T  # [QF, E]
    wo = np.ascontiguousarray(
        wo.reshape(c.NPAIR, 128, c.E).transpose(1, 0, 2)).astype(
            ml_dtypes.bfloat16)
    mt = np.where(mask[0, 0, :c.KC, :c.KC].T != 0, 0.0,
                  c.MASK_NEG).astype(ml_dtypes.bfloat16)
    mt2 = np.ascontiguousarray(
        np.repeat(mt[:, None, :], 2, axis=1))  # [128, 2, KC]
    xtr = np.ascontiguousarray(
        xb.T.reshape(c.DC, 128, c.NT).transpose(1, 0, 2)).astype(
            ml_dtypes.bfloat16)
    return {
        "xT": xtr,
        "wqk": wqk_t,
        "wk8": wk8_t,
        "wv": wv_t,
        "wout": wo,
        "mtri": mt2,
        "iden": np.eye(128, dtype=ml_dtypes.bfloat16),
    }


_CACHE = {}


def run_sharded(x, mask, w_qkv, w_out, trace=False, trace_cores=None):
    """Shard inputs over 8 cores, run the bass kernel, gather full output.
    Returns (out [B,N,D] f32, BassKernelResults)."""
    # the axon PJRT backend is required for execution; guard against a
    # caller environment that overrode JAX_PLATFORMS before jax init
    if 'jax' not in sys.modules and 'axon' not in os.environ.get(
            'JAX_PLATFORMS', 'axon'):
        os.environ['JAX_PLATFORMS'] = 'axon'
    from concourse.bass_utils import run_bass_kernel_spmd

    cfg = Cfg()
    B = x.shape[0]
    n_cores = 2 * B
    if 'nc' not in _CACHE:
        _CACHE['nc'] = build(cfg)
    nc = _CACHE['nc']

    x = np.asarray(x, np.float32)
    mask = np.asarray(mask)
    w_qkv = np.asarray(w_qkv, np.float32)
    w_out = np.asarray(w_out, np.float32)

    in_maps = []
    for core in range(n_cores):
        b, half = core // 2, core % 2
        in_maps.append(make_core_inputs(x[b], w_qkv, w_out, mask, cfg, half))

    if trace:
        _install_ntff_hook()
    res = run_bass_kernel_spmd(
        nc, in_maps, core_ids=list(range(n_cores)), trace=trace,
        trace_cores=trace_cores)
    outs = []
    for b in range(B):
        outs.append(res.results[2 * b]["out"].astype(np.float64)
                    + res.results[2 * b + 1]["out"].astype(np.float64))
    return np.stack(outs).astype(np.float32), res


def kernel(x, mask, w_qkv, w_out):
    out, _ = run_sharded(x, mask, w_qkv, w_out, trace=False)
    return out



# revision 42
# speedup vs baseline: 1.0370x; 1.0370x over previous
"""Self-contained TRN2 Bass kernel for nn_Attention (B=4, N=2048, D=1024, H=16).

Sharding: 8 NeuronCores, core c = (batch b = c//2, head-half = c%2).
Each core computes causal attention for its batch and 8 of 16 heads plus the
row-parallel half of the output projection; the host sums the two half-partials
per batch.

Per-core pipeline (all on-device, Tile-scheduled):
  q-proj bf16; k-proj fp8e4m3 DoubleRow (64x weight scale folded into the
    host-side q scale); v-proj bf16 -> V [token, feat] (+ones col)
  S^T chunk [128 k-tok, 512 q-tok] = kT-part @ qT   (bf16, f32 psum)
  causal mask added on the diagonal blocks via one paired PE matmul
  exp on ScalarE -> P bf16
  O^T + softmax denominator via P^T @ [V | 1]
  1/den: both heads' den rows folded to a [128, 8] tile by DMA, DVE
    reciprocal there, DRAM bounce for the partition broadcast
  partial out [NT, E] = OT.T @ woutT  (bf16 matmuls, f32 psum); the last
    stores split across engines to keep the tail off a single DMA queue
"""

import os
import sys
import types
from contextlib import ExitStack
from dataclasses import dataclass

for _p in ('/opt/trn_rl_repo', '/root/.axon_site/_ro/trn_rl_repo'):
    if os.path.isdir(_p) and _p not in sys.path:
        sys.path.append(_p)

import numpy as np
import ml_dtypes

import concourse.bass as bass
import concourse.mybir as mybir
import concourse.tile as tile
from concourse import bacc

F32 = mybir.dt.float32
F32R = mybir.dt.float32r
BF16 = mybir.dt.bfloat16
F8 = mybir.dt.float8e4


# ---------------------------------------------------------------- harness fixes
def _install_ntff_hook():
    """Register the axon NTFF profile hook that trn_boot skips when the
    container's antenv stub lacks axon_hooks (needed only for trace=True)."""
    if 'antenv.axon_hooks' in sys.modules:
        return
    try:
        import antenv
        mod = types.ModuleType('antenv.axon_hooks')
        _hook = [None]
        mod.set_axon_ntff_profile_hook = lambda h: _hook.__setitem__(0, h)
        mod.get_axon_ntff_profile_hook = lambda: _hook[0]
        sys.modules['antenv.axon_hooks'] = mod
        antenv.axon_hooks = mod
        from trn_agent_boot.trn_boot import _ntff_profile_via_ctypes
        so = '/opt/axon/libaxon_pjrt.so'
        if os.path.exists(so):
            hook = _ntff_profile_via_ctypes(so)
            if hook is not None:
                mod.set_axon_ntff_profile_hook(hook)
    except Exception:
        pass


def _patch_tile_drain():
    """walrus TPB_CTRL encodes <=2 sync waits; Tile's tail drain can carry
    more. Split extras onto single-wait nops (sequentially equivalent)."""
    import concourse.tile as tile_mod
    if getattr(tile_mod.TileContext, '_drain_patched', False):
        return
    from concourse.tile import ScopedClock

    def _drain_and_barrier(self, tick_clock, wait_clock):
        nc = self.nc
        drain_inst = nc.sync.drain()
        wait_clock.add_sem_waits(
            drain_inst.ins, ScopedClock({None: tick_clock.global_clock}))
        si = drain_inst.ins.sync_info
        if si is not None and si.on_wait and len(si.on_wait) > 1:
            waits = list(si.on_wait)
            drain_inst.ins.sync_info = mybir.SyncInfo(
                on_wait=waits[:1], on_update=list(si.on_update or []))
            for w in waits[1:]:
                nop = nc.sync.nop(nofuse=True)
                nop.ins.sync_info = mybir.SyncInfo(on_wait=[w], on_update=[])
        nc.all_engine_barrier()
        popped = nc._tile_sem_poison_stack.pop()
        assert popped is self._sem_poison
        nc.clear_and_free_semaphores(list(self.sems.allocated().values()))
        nc.all_engine_barrier()

    tile_mod.TileContext._drain_and_barrier = _drain_and_barrier
    tile_mod.TileContext._drain_patched = True


# ---------------------------------------------------------------- kernel build
@dataclass(frozen=True)
class Cfg:
    NT: int = 2048   # tokens
    D: int = 1024    # model dim
    HH: int = 8      # heads per core
    DH: int = 64     # head dim
    E: int = 1024    # output features
    QC: int = 512    # q-chunk (free dim of score tiles)
    KC: int = 128    # k-tile (partition dim of score tiles)
    MASK_NEG: float = -1e30
    PT_BUFS: int = 8
    PSS_BUFS: int = 2
    PSO_BUFS: int = 2
    OT_BUFS: int = 8
    DEN_BUFS: int = 6
    XT_BUFS: int = 2
    PSA_BUFS: int = 4

    @property
    def DC(self): return self.D // 128
    @property
    def NKT(self): return self.NT // self.KC
    @property
    def NQC(self): return self.NT // self.QC
    @property
    def NPAIR(self): return self.HH // 2
    @property
    def QF(self): return self.HH * self.DH
    @property
    def VW(self): return self.DH + 1


def build(cfg: Cfg) -> bass.Bass:
    _patch_tile_drain()
    nc = bacc.Bacc('TRN2', target_bir_lowering=False)
    c = cfg
    assert c.QC % c.KC == 0 and c.NT % c.QC == 0 and c.D % 128 == 0
    assert c.DH * 2 == c.KC
    JPT = c.QC // c.KC

    xT = nc.declare_dram_parameter("xT", [128, c.DC, c.NT], BF16, isOutput=False)
    wq8 = nc.declare_dram_parameter("wq8", [128, c.DC // 2, 2, c.QF], F8,
                                    isOutput=False)
    wk8 = nc.declare_dram_parameter("wk8", [128, c.DC // 2, 2, c.QF], F8,
                                    isOutput=False)
    wv = nc.declare_dram_parameter("wv", [128, c.DC, c.QF], BF16, isOutput=False)
    wout = nc.declare_dram_parameter("wout", [128, c.NPAIR, c.E], BF16, isOutput=False)
    mtri = nc.declare_dram_parameter("mtri", [128, 2, c.KC], BF16, isOutput=False)
    iden = nc.declare_dram_parameter("iden", [128, 128], BF16, isOutput=False)
    out = nc.declare_dram_parameter("out", [c.NT, c.E], F32, isOutput=True)

    with tile.TileContext(nc) as tc, ExitStack() as ctx:
        const = ctx.enter_context(tc.tile_pool(name="const", bufs=1))
        persist = ctx.enter_context(tc.tile_pool(name="persist", bufs=1))

        ones64 = const.tile([c.VW, c.DH], F32)
        nc.vector.memset(ones64[c.DH:c.VW, :], 1.0)
        mtri_sb = const.tile([128, 2, c.KC], BF16)
        iden_sb = const.tile([128, 128], BF16)
        wv_sb = const.tile([128, c.DC, c.QF], BF16)
        wq8_sb = const.tile([128, c.DC // 2, 2, c.QF], F8)
        wk8_sb = const.tile([128, c.DC // 2, 2, c.QF], F8)
        wout_sb = const.tile([128, c.NPAIR, c.E], BF16)

        qk_sb = [persist.tile([128, c.NT], BF16, tag=f"qk{e}", name=f"qk{e}")
                 for e in range(2 * c.NPAIR)]
        V_sb = persist.tile([128, c.NKT, c.HH, c.VW], BF16, tag="V", name="V_sb")
        nc.vector.memset(V_sb[:, :, :, c.DH], 1.0)
        OT_sb = [persist.tile([128, c.NT], BF16, tag=f"ot{p}", name=f"ot{p}")
                 for p in range(c.NPAIR)]

        xt2 = persist.tile([128, c.DC, c.NT], BF16, tag="xt", name="xt2")
        x8 = persist.tile([128, c.DC, c.NT], F8, tag="x8", name="x8")
        # Input DMA schedule.  Fine-grained transfers spread across DMA
        # queues (a single big DMA rides one engine at ~1/16 of the HBM
        # bandwidth).  Critical path on sync: x chunk 0 + wv gate Phase A,
        # then wqk's pair-0 columns gate the q/k prologue.  Everything
        # else is issued from the idle GpSimd engine's software DGE.
        ch = c.QC
        for dc in range(c.DC):
            # x chunk 0 + wv split across the sync and scalar DGEs so the
            # triggers issue in parallel; dc 0/1 transfers are halved so
            # Phase A's first matmuls are not gated on a full 128KB
            # single-engine transfer
            eng = nc.sync if dc % 2 == 0 else nc.scalar
            if dc < 2:
                h = ch // 2
                eng.dma_start(out=xt2[:, dc, 0:h], in_=xT[:, dc, 0:h])
                eng.dma_start(out=xt2[:, dc, h:ch], in_=xT[:, dc, h:ch])
                hf = c.QF // 2
                eng.dma_start(out=wv_sb[:, dc, 0:hf], in_=wv[:, dc, 0:hf])
                eng.dma_start(out=wv_sb[:, dc, hf:], in_=wv[:, dc, hf:])
            else:
                eng.dma_start(out=xt2[:, dc, 0:ch], in_=xT[:, dc, 0:ch])
                eng.dma_start(out=wv_sb[:, dc, :], in_=wv[:, dc, :])
        nc.sync.dma_start(out=mtri_sb[:], in_=mtri[:])
        nc.scalar.dma_start(out=iden_sb[:], in_=iden[:])
        for dcp in range(c.DC // 2):
            # fp8 q and k weights (both projections run in fp8 DoubleRow
            # off a casted copy of x; descale folds into the exp scale)
            eng = nc.sync if dcp % 2 == 0 else nc.scalar
            eng.dma_start(out=wq8_sb[:, dcp], in_=wq8[:, dcp])
            eng.dma_start(out=wk8_sb[:, dcp], in_=wk8[:, dcp])
        # x chunk 0 casted to fp8 (gpsimd DGE casts in flight) — needed by
        # the k prologue first
        for d2 in range(0, c.DC, 2):
            nc.gpsimd.dma_start(
                out=x8[:, d2:d2 + 2, 0:ch], in_=xT[:, d2:d2 + 2, 0:ch])
        for dc in range(c.DC):
            # x chunk 1 (needed by pair-0 fillers early)
            eng = nc.sync if dc % 2 == 0 else nc.scalar
            eng.dma_start(out=xt2[:, dc, ch:2 * ch], in_=xT[:, dc, ch:2 * ch])
        # interleave the casted-x and bf16-x chunk streams by need time
        for cs in range(ch, c.NT, ch):
            for d2 in range(0, c.DC, 2):
                nc.gpsimd.dma_start(
                    out=x8[:, d2:d2 + 2, cs:cs + ch],
                    in_=xT[:, d2:d2 + 2, cs:cs + ch])
            if cs + ch < c.NT:
                for d2 in range(0, c.DC, 2):
                    nc.gpsimd.dma_start(
                        out=xt2[:, d2:d2 + 2, cs + ch:cs + 2 * ch],
                        in_=xT[:, d2:d2 + 2, cs + ch:cs + 2 * ch])
        for pr in range(c.NPAIR):
            nc.gpsimd.dma_start(
                out=wout_sb[:, pr, :], in_=wout[:, pr, :])

        # ---------------- Phase A: v-projection (first q-chunk only;
        # the rest is emitted as filler work inside pair 0) ----------------
        NTT0 = c.QC // 128
        with tc.tile_pool(name="ps_a", bufs=c.PSA_BUFS, space="PSUM") as ps_a:
            # dc-outer order so each arriving x/wv chunk is consumed
            # immediately across all four psum tiles
            psvs = [ps_a.tile([128, c.QF], F32, tag="ps", name=f"psv{nt}")
                    for nt in range(min(NTT0, c.NKT))]
            for dc in range(c.DC):
                for nt, psv in enumerate(psvs):
                    nc.tensor.matmul(
                        psv[:],
                        lhsT=xt2[:, dc, nt * 128:(nt + 1) * 128],
                        rhs=wv_sb[:, dc, :],
                        start=(dc == 0), stop=(dc == c.DC - 1),
                    )
            for nt, psv in enumerate(psvs):
                nc.vector.tensor_copy(
                    out=V_sb[:, nt, :, 0:c.DH],
                    in_=psv[:].rearrange("p (h f) -> p h f", h=c.HH),
                )

        # ---------------- Phase B: attention ----------------
        with (
            tc.tile_pool(name="pt", bufs=c.PT_BUFS) as pt_pool,
            tc.tile_pool(name="otst", bufs=c.OT_BUFS) as ot_pool,
            tc.tile_pool(name="den", bufs=c.DEN_BUFS) as den_pool,
            tc.tile_pool(name="dend", bufs=4, space="DRAM") as dend_pool,
            tc.tile_pool(name="ps_s", bufs=c.PSS_BUFS, space="PSUM") as ps_s,
            tc.tile_pool(name="ps_o", bufs=c.PSO_BUFS, space="PSUM") as ps_o,
            tc.tile_pool(name="ps_f", bufs=2, space="PSUM") as ps_f,
            tc.tile_pool(name="osbB", bufs=3) as outB_pool,
        ):
            def evac_stage1(psO):
                ocps = []
                for h2 in range(2):
                    # evacuate psum to SBUF immediately to free the bank
                    # (GPSIMD cannot read PSUM, so both stay on DVE)
                    ocp = ot_pool.tile([c.VW, c.QC], F32, tag="ocp",
                                       name="ocp", bufs=6)
                    nc.vector.tensor_copy(out=ocp[:], in_=psO[h2][:])
                    ocps.append(ocp)
                return ocps

            def evac_stage2(p, t, ocps):
                qsl_full = slice(t * c.QC, (t + 1) * c.QC)
                # Fold both heads' denominator rows [1, QC] into a [128, *]
                # square via DMA so the DVE reciprocal runs across all 128
                # lanes (~130ns instead of ~3.3us on one lane), then bounce
                # through DRAM for the partition broadcast.  Both hops use
                # identical element orderings, so positions are preserved.
                QW = c.QC // 128
                den_sq = den_pool.tile([128, 2, QW], F32, tag="dsq",
                                       name="den_sq", bufs=3)
                for h2 in range(2):
                    nc.sync.dma_start(
                        out=den_sq[:, h2, :], in_=ocps[h2][c.DH:c.VW, :])
                rec_sq = den_pool.tile([128, 2, QW], F32, tag="rsq",
                                       name="rec_sq", bufs=3)
                nc.vector.reciprocal(out=rec_sq[:], in_=den_sq[:])
                rec_d = dend_pool.tile([2, c.QC], F32, tag="dend",
                                       name="rec_d")
                for h2 in range(2):
                    nc.sync.dma_start(
                        out=rec_d[h2], in_=rec_sq[:, h2, :])
                for h2 in range(2):
                    ocp = ocps[h2]
                    divB = den_pool.tile([c.DH, c.QC], F32, tag="div",
                                         name="divB")
                    nc.sync.dma_start(
                        out=divB[:], in_=rec_d[h2].partition_broadcast(c.DH))
                    if h2 == 0:
                        nc.vector.tensor_tensor(
                            out=OT_sb[p][0:c.DH, qsl_full], in0=ocp[0:c.DH, :],
                            in1=divB[:], op=mybir.AluOpType.mult,
                        )
                    else:
                        ot_st = ot_pool.tile([c.DH, c.QC], BF16, tag="ot",
                                             name="ot_st")
                        nc.gpsimd.tensor_tensor(
                            out=ot_st[:], in0=ocp[0:c.DH, :], in1=divB[:],
                            op=mybir.AluOpType.mult,
                        )
                        nc.sync.dma_start(
                            out=OT_sb[p][c.DH:2 * c.DH, qsl_full],
                            in_=ot_st[:])

            def emit_proj_chunk(nt, ec, split=False):
                esl = slice(ec * c.QC, (ec + 1) * c.QC)
                psP = ps_f.tile([128, c.QC], F32, tag="f", name="psP")
                for pr in range(c.NPAIR):
                    nc.tensor.matmul(
                        psP[:],
                        lhsT=OT_sb[pr][:, nt * 128:(nt + 1) * 128],
                        rhs=wout_sb[:, pr, esl],
                        start=(pr == 0), stop=(pr == c.NPAIR - 1),
                    )
                o_sb = outB_pool.tile([128, c.QC], F32, tag="ob", name="o_sb")
                nc.vector.tensor_copy(out=o_sb[:], in_=psP[:])
                if not split:
                    nc.sync.dma_start(
                        out=out[nt * 128:(nt + 1) * 128, esl], in_=o_sb[:])
                else:
                    # the run's last stores: 4-way partition split across
                    # three trigger engines so the 256KB transfer does not
                    # ride a single 22GB/s DMA engine on the critical tail
                    for i, eng in enumerate(
                            (nc.sync, nc.scalar, nc.gpsimd, nc.sync)):
                        ps = slice(i * 32, (i + 1) * 32)
                        eng.dma_start(
                            out=out[nt * 128 + i * 32:
                                    nt * 128 + (i + 1) * 32, esl],
                            in_=o_sb[ps, :])

            def emit_qk_chunk(pp, ci):
                is_k = ci // c.NQC
                e = (pp, c.NPAIR + pp)[is_k]
                ncc = ci % c.NQC
                nsl = slice(ncc * c.QC, (ncc + 1) * c.QC)
                psqk = ps_f.tile([128, c.QC], F32, tag="f", name="psqk")
                w8 = wk8_sb if is_k else wq8_sb
                # fp8 DoubleRow: two D-chunks per pass, 0.5 cycles/row
                for dcp in range(c.DC // 2):
                    nc.tensor.matmul(
                        psqk[:],
                        lhsT=w8[:, dcp, :, pp * 128:(pp + 1) * 128],
                        rhs=x8[:, 2 * dcp:2 * dcp + 2, nsl],
                        start=(dcp == 0), stop=(dcp == c.DC // 2 - 1),
                        perf_mode=mybir.MatmulPerfMode.DoubleRow,
                    )
                nc.vector.tensor_copy(out=qk_sb[e][:, nsl], in_=psqk[:])

            def emit_v_chunk(nt):
                psv = ps_f.tile([128, c.QF], F32, tag="f", name="psvf")
                for dc in range(c.DC):
                    nc.tensor.matmul(
                        psv[:],
                        lhsT=xt2[:, dc, nt * 128:(nt + 1) * 128],
                        rhs=wv_sb[:, dc, :],
                        start=(dc == 0), stop=(dc == c.DC - 1),
                    )
                nc.vector.tensor_copy(
                    out=V_sb[:, nt, :, 0:c.DH],
                    in_=psv[:].rearrange("p (h f) -> p h f", h=c.HH),
                )

            pending2 = None
            # prologue: only the first q-chunk's columns of q and k
            emit_qk_chunk(0, 0)
            emit_qk_chunk(0, c.NQC)
            for p in range(c.NPAIR):
                q_t, k_t = qk_sb[p], qk_sb[c.NPAIR + p]
                for t in range(c.NQC):
                    njt = JPT * t + JPT
                    psO = [ps_o.tile([c.VW, c.QC], F32, tag="o", name=f"psO{_h}")
                           for _h in range(2)]
                    def emit_pv(items):
                        for (h2_, pt_, lo_, j_) in items:
                            nc.tensor.matmul(
                                psO[h2_][:, lo_:],
                                lhsT=V_sb[:, j_, 2 * p + h2_, :],
                                rhs=pt_[:, h2_, lo_:],
                                start=(j_ == 0), stop=(j_ == njt - 1),
                            )

                    pipe = []
                    for j in range(njt):
                        off = j * c.KC - t * c.QC
                        band = off >= 0
                        lo = max(off, 0)
                        jsl = slice(j * c.KC, (j + 1) * c.KC)
                        qsl = slice(t * c.QC + lo, (t + 1) * c.QC)
                        # both heads' scores into one 2-bank psum tile
                        psS = ps_s.tile([128, 2, c.QC], F32, tag="s", name="psS")
                        for h2 in range(2):
                            hsl = slice(h2 * c.DH, (h2 + 1) * c.DH)
                            nc.tensor.matmul(
                                psS[:, h2, lo:], lhsT=k_t[hsl, jsl],
                                rhs=q_t[hsl, qsl], start=True, stop=(not band),
                            )
                        if band:
                            # causal mask add on PE for both heads at once:
                            # psum += I.T @ [mtri | mtri]
                            nc.tensor.matmul(
                                psS[:, :, off:off + c.KC], lhsT=iden_sb[:],
                                rhs=mtri_sb[:], start=False, stop=True,
                                skip_group_check=True,
                            )
                        pt_t = pt_pool.tile([128, 2, c.QC], BF16, tag="pt",
                                            name="pt_t")
                        nc.scalar.activation(
                            out=pt_t[:, :, lo:],
                            in_=psS[:, :, lo:],
                            func=mybir.ActivationFunctionType.Exp,
                            scale=1.0 / 32768.0,
                        )
                        pipe.append([(0, pt_t, lo, j), (1, pt_t, lo, j)])
                        if len(pipe) > 3:
                            emit_pv(pipe.pop(0))
                    # interleave filler work (next pair's qk projection,
                    # or output-projection chunks during the last pair) with
                    # the PV pipe flush so the PE has matmuls to run while
                    # ScalarE finishes the tail exps
                    NTT = c.QC // 128
                    if p == 0:
                        fillers = []
                        if t + 1 < c.NQC:
                            # next q-chunk's V rows and q/k columns
                            fillers += [
                                lambda nt_=nt_: emit_v_chunk(nt_)
                                for nt_ in range((t + 1) * NTT,
                                                 min((t + 2) * NTT, c.NKT))]
                            fillers += [
                                lambda ci=ci: emit_qk_chunk(0, ci)
                                for ci in (t + 1, c.NQC + t + 1)]
                        fillers += [lambda i=i: emit_qk_chunk(1, 2 * t + i)
                                    for i in range(2)]
                    elif p + 1 < c.NPAIR:
                        fillers = [lambda i=i: emit_qk_chunk(p + 1, 2 * t + i)
                                   for i in range(2)]
                    elif t >= 2:
                        # two-chunk lag keeps the filler projections clear
                        # of the still-in-flight evac chain of chunk t-1
                        tp = t - 2
                        fillers = [
                            lambda nt_=nt_, ec_=ec_: emit_proj_chunk(nt_, ec_)
                            for nt_ in range(tp * NTT, (tp + 1) * NTT)
                            for ec_ in range(c.E // c.QC)]
                    else:
                        fillers = []
                    while fillers or pipe:
                        if fillers:
                            fillers.pop(0)()
                        if pipe:
                            emit_pv(pipe.pop(0))
                    ocps = evac_stage1(psO)
                    if pending2 is not None:
                        evac_stage2(*pending2)
                        pending2 = None
                    if p == c.NPAIR - 1:
                        # last pair: evacuate immediately so the output
                        # projection can chase with only one chunk of lag
                        evac_stage2(p, t, ocps)
                    else:
                        pending2 = (p, t, ocps)

            if pending2 is not None:
                evac_stage2(*pending2)

            # final q-chunks' output projection, still inside the same pool
            # scope (no drain barrier before it); chunk NQC-2 is ready
            # immediately and overlaps the last chunk's evac chain
            NTTf = c.QC // 128
            for nt in range((c.NQC - 2) * NTTf, c.NT // 128):
                for ec in range(c.E // c.QC):
                    emit_proj_chunk(nt, ec, split=(nt >= c.NT // 128 - 2))

    nc.compile()
    return nc


# ---------------------------------------------------------------- host side
def make_core_inputs(xb, w_qkv, w_out, mask, cfg, half):
    c = cfg
    D = c.D
    K8_SCALE = 64.0  # fp8 weight scale; 1/(64*64*sqrt(DH)) folds into exp
    heads = range(half * c.HH, (half + 1) * c.HH)
    q_rows = np.concatenate(
        [w_qkv[h * c.DH:(h + 1) * c.DH, :] for h in heads])
    k_rows = np.concatenate(
        [w_qkv[D + h * c.DH:D + (h + 1) * c.DH, :] for h in heads])
    v_rows = np.concatenate(
        [w_qkv[2 * D + h * c.DH:2 * D + (h + 1) * c.DH, :] for h in heads])

    def pack8(rows):
        # [p, dcp, o, f] = 64*w[f, (2*dcp+o)*128 + p] in fp8e4m3
        return np.ascontiguousarray(
            (rows.T * K8_SCALE).reshape(c.DC // 2, 2, 128, c.QF)
            .transpose(2, 0, 1, 3)).astype(ml_dtypes.float8_e4m3fn)

    wq8_t = pack8(q_rows)
    wk8_t = pack8(k_rows)
    wv_t = np.ascontiguousarray(
        v_rows.T.reshape(c.DC, 128, c.QF).transpose(1, 0, 2)).astype(
            ml_dtypes.bfloat16)
    wo = w_out[:, half * c.QF:(half + 1) * c.QF].T  # [QF, E]
    wo = np.ascontiguousarray(
        wo.reshape(c.NPAIR, 128, c.E).transpose(1, 0, 2)).astype(
            ml_dtypes.bfloat16)
    mt = np.where(mask[0, 0, :c.KC, :c.KC].T != 0, 0.0,
                  c.MASK_NEG).astype(ml_dtypes.bfloat16)
    mt2 = np.ascontiguousarray(
        np.repeat(mt[:, None, :], 2, axis=1))  # [128, 2, KC]
    xtr = np.ascontiguousarray(
        xb.T.reshape(c.DC, 128, c.NT).transpose(1, 0, 2)).astype(
            ml_dtypes.bfloat16)
    return {
        "xT": xtr,
        "wq8": wq8_t,
        "wk8": wk8_t,
        "wv": wv_t,
        "wout": wo,
        "mtri": mt2,
        "iden": np.eye(128, dtype=ml_dtypes.bfloat16),
    }


_CACHE = {}


def run_sharded(x, mask, w_qkv, w_out, trace=False, trace_cores=None):
    """Shard inputs over 8 cores, run the bass kernel, gather full output.
    Returns (out [B,N,D] f32, BassKernelResults)."""
    # the axon PJRT backend is required for execution; guard against a
    # caller environment that overrode JAX_PLATFORMS before jax init
    if 'jax' not in sys.modules and 'axon' not in os.environ.get(
            'JAX_PLATFORMS', 'axon'):
        os.environ['JAX_PLATFORMS'] = 'axon'
    from concourse.bass_utils import run_bass_kernel_spmd

    cfg = Cfg()
    B = x.shape[0]
    n_cores = 2 * B
    if 'nc' not in _CACHE:
        _CACHE['nc'] = build(cfg)
    nc = _CACHE['nc']

    x = np.asarray(x, np.float32)
    mask = np.asarray(mask)
    w_qkv = np.asarray(w_qkv, np.float32)
    w_out = np.asarray(w_out, np.float32)

    in_maps = []
    for core in range(n_cores):
        b, half = core // 2, core % 2
        in_maps.append(make_core_inputs(x[b], w_qkv, w_out, mask, cfg, half))

    if trace:
        _install_ntff_hook()
    res = run_bass_kernel_spmd(
        nc, in_maps, core_ids=list(range(n_cores)), trace=trace,
        trace_cores=trace_cores)
    outs = []
    for b in range(B):
        outs.append(res.results[2 * b]["out"].astype(np.float64)
                    + res.results[2 * b + 1]["out"].astype(np.float64))
    return np.stack(outs).astype(np.float32), res


def kernel(x, mask, w_qkv, w_out):
    out, _ = run_sharded(x, mask, w_qkv, w_out, trace=False)
    return out



# revision 43
# speedup vs baseline: 1.0416x; 1.0045x over previous
"""Self-contained TRN2 Bass kernel for nn_Attention (B=4, N=2048, D=1024, H=16).

Sharding: 8 NeuronCores, core c = (batch b = c//2, head-half = c%2).
Each core computes causal attention for its batch and 8 of 16 heads plus the
row-parallel half of the output projection; the host sums the two half-partials
per batch.

Per-core pipeline (all on-device, Tile-scheduled):
  q-proj bf16; k-proj fp8e4m3 DoubleRow (64x weight scale folded into the
    host-side q scale); v-proj bf16 -> V [token, feat] (+ones col)
  S^T chunk [128 k-tok, 512 q-tok] = kT-part @ qT   (bf16, f32 psum)
  causal mask added on the diagonal blocks via one paired PE matmul
  exp on ScalarE -> P bf16
  O^T + softmax denominator via P^T @ [V | 1]
  1/den: both heads' den rows folded to a [128, 8] tile by DMA, DVE
    reciprocal there, DRAM bounce for the partition broadcast
  partial out [NT, E] = OT.T @ woutT  (bf16 matmuls, f32 psum); the last
    stores split across engines to keep the tail off a single DMA queue
"""

import os
import sys
import types
from contextlib import ExitStack
from dataclasses import dataclass

for _p in ('/opt/trn_rl_repo', '/root/.axon_site/_ro/trn_rl_repo'):
    if os.path.isdir(_p) and _p not in sys.path:
        sys.path.append(_p)

import numpy as np
import ml_dtypes

import concourse.bass as bass
import concourse.mybir as mybir
import concourse.tile as tile
from concourse import bacc

F32 = mybir.dt.float32
F32R = mybir.dt.float32r
BF16 = mybir.dt.bfloat16
F8 = mybir.dt.float8e4


# ---------------------------------------------------------------- harness fixes
def _install_ntff_hook():
    """Register the axon NTFF profile hook that trn_boot skips when the
    container's antenv stub lacks axon_hooks (needed only for trace=True)."""
    if 'antenv.axon_hooks' in sys.modules:
        return
    try:
        import antenv
        mod = types.ModuleType('antenv.axon_hooks')
        _hook = [None]
        mod.set_axon_ntff_profile_hook = lambda h: _hook.__setitem__(0, h)
        mod.get_axon_ntff_profile_hook = lambda: _hook[0]
        sys.modules['antenv.axon_hooks'] = mod
        antenv.axon_hooks = mod
        from trn_agent_boot.trn_boot import _ntff_profile_via_ctypes
        so = '/opt/axon/libaxon_pjrt.so'
        if os.path.exists(so):
            hook = _ntff_profile_via_ctypes(so)
            if hook is not None:
                mod.set_axon_ntff_profile_hook(hook)
    except Exception:
        pass


def _patch_tile_drain():
    """walrus TPB_CTRL encodes <=2 sync waits; Tile's tail drain can carry
    more. Split extras onto single-wait nops (sequentially equivalent)."""
    import concourse.tile as tile_mod
    if getattr(tile_mod.TileContext, '_drain_patched', False):
        return
    from concourse.tile import ScopedClock

    def _drain_and_barrier(self, tick_clock, wait_clock):
        nc = self.nc
        drain_inst = nc.sync.drain()
        wait_clock.add_sem_waits(
            drain_inst.ins, ScopedClock({None: tick_clock.global_clock}))
        si = drain_inst.ins.sync_info
        if si is not None and si.on_wait and len(si.on_wait) > 1:
            waits = list(si.on_wait)
            drain_inst.ins.sync_info = mybir.SyncInfo(
                on_wait=waits[:1], on_update=list(si.on_update or []))
            for w in waits[1:]:
                nop = nc.sync.nop(nofuse=True)
                nop.ins.sync_info = mybir.SyncInfo(on_wait=[w], on_update=[])
        nc.all_engine_barrier()
        popped = nc._tile_sem_poison_stack.pop()
        assert popped is self._sem_poison
        nc.clear_and_free_semaphores(list(self.sems.allocated().values()))
        nc.all_engine_barrier()

    tile_mod.TileContext._drain_and_barrier = _drain_and_barrier
    tile_mod.TileContext._drain_patched = True


# ---------------------------------------------------------------- kernel build
@dataclass(frozen=True)
class Cfg:
    NT: int = 2048   # tokens
    D: int = 1024    # model dim
    HH: int = 8      # heads per core
    DH: int = 64     # head dim
    E: int = 1024    # output features
    QC: int = 512    # q-chunk (free dim of score tiles)
    KC: int = 128    # k-tile (partition dim of score tiles)
    MASK_NEG: float = -1e30
    PT_BUFS: int = 8
    PSS_BUFS: int = 2
    PSO_BUFS: int = 2
    OT_BUFS: int = 8
    DEN_BUFS: int = 6
    XT_BUFS: int = 2
    PSA_BUFS: int = 4

    @property
    def DC(self): return self.D // 128
    @property
    def NKT(self): return self.NT // self.KC
    @property
    def NQC(self): return self.NT // self.QC
    @property
    def NPAIR(self): return self.HH // 2
    @property
    def QF(self): return self.HH * self.DH
    @property
    def VW(self): return self.DH + 1


def build(cfg: Cfg) -> bass.Bass:
    _patch_tile_drain()
    nc = bacc.Bacc('TRN2', target_bir_lowering=False)
    c = cfg
    assert c.QC % c.KC == 0 and c.NT % c.QC == 0 and c.D % 128 == 0
    assert c.DH * 2 == c.KC
    JPT = c.QC // c.KC

    xT = nc.declare_dram_parameter("xT", [128, c.DC, c.NT], BF16, isOutput=False)
    wq8 = nc.declare_dram_parameter("wq8", [128, c.DC // 2, 2, c.QF], F8,
                                    isOutput=False)
    wk8 = nc.declare_dram_parameter("wk8", [128, c.DC // 2, 2, c.QF], F8,
                                    isOutput=False)
    wv = nc.declare_dram_parameter("wv", [128, c.DC, c.QF], BF16, isOutput=False)
    wout = nc.declare_dram_parameter("wout", [128, c.NPAIR, c.E], BF16, isOutput=False)
    mtri = nc.declare_dram_parameter("mtri", [128, 2, c.KC], BF16, isOutput=False)
    iden = nc.declare_dram_parameter("iden", [128, 128], BF16, isOutput=False)
    out = nc.declare_dram_parameter("out", [c.NT, c.E], F32, isOutput=True)

    with tile.TileContext(nc) as tc, ExitStack() as ctx:
        const = ctx.enter_context(tc.tile_pool(name="const", bufs=1))
        persist = ctx.enter_context(tc.tile_pool(name="persist", bufs=1))

        ones64 = const.tile([c.VW, c.DH], F32)
        nc.vector.memset(ones64[c.DH:c.VW, :], 1.0)
        mtri_sb = const.tile([128, 2, c.KC], BF16)
        iden_sb = const.tile([128, 128], BF16)
        wv_sb = const.tile([128, c.DC, c.QF], BF16)
        wq8_sb = const.tile([128, c.DC // 2, 2, c.QF], F8)
        wk8_sb = const.tile([128, c.DC // 2, 2, c.QF], F8)
        wout_sb = const.tile([128, c.NPAIR, c.E], BF16)

        qk_sb = [persist.tile([128, c.NT], BF16, tag=f"qk{e}", name=f"qk{e}")
                 for e in range(2 * c.NPAIR)]
        V_sb = persist.tile([128, c.NKT, c.HH, c.VW], BF16, tag="V", name="V_sb")
        nc.vector.memset(V_sb[:, :, :, c.DH], 1.0)
        OT_sb = [persist.tile([128, c.NT], BF16, tag=f"ot{p}", name=f"ot{p}")
                 for p in range(c.NPAIR)]

        xt2 = persist.tile([128, c.DC, c.NT], BF16, tag="xt", name="xt2")
        x8 = persist.tile([128, c.DC, c.NT], F8, tag="x8", name="x8")
        # Input DMA schedule.  Fine-grained transfers spread across DMA
        # queues (a single big DMA rides one engine at ~1/16 of the HBM
        # bandwidth).  Critical path on sync: x chunk 0 + wv gate Phase A,
        # then wqk's pair-0 columns gate the q/k prologue.  Everything
        # else is issued from the idle GpSimd engine's software DGE.
        ch = c.QC
        # fp8 q/k weights + casted x chunk 0 gate the q/k prologue: they
        # are small (0.75MB) so the PE starts matmuls ~5us earlier than
        # waiting for the 2MB xt0+wv set
        for dcp in range(c.DC // 2):
            eng = nc.sync if dcp % 2 == 0 else nc.scalar
            eng.dma_start(out=wq8_sb[:, dcp], in_=wq8[:, dcp])
            eng.dma_start(out=wk8_sb[:, dcp], in_=wk8[:, dcp])
        for d2 in range(0, c.DC, 2):
            nc.gpsimd.dma_start(
                out=x8[:, d2:d2 + 2, 0:ch], in_=xT[:, d2:d2 + 2, 0:ch])
        for dc in range(c.DC):
            # x chunk 0 + wv (v-projection) split across the sync and
            # scalar DGEs so the triggers issue in parallel
            eng = nc.sync if dc % 2 == 0 else nc.scalar
            eng.dma_start(out=xt2[:, dc, 0:ch], in_=xT[:, dc, 0:ch])
            eng.dma_start(out=wv_sb[:, dc, :], in_=wv[:, dc, :])
        nc.sync.dma_start(out=mtri_sb[:], in_=mtri[:])
        nc.scalar.dma_start(out=iden_sb[:], in_=iden[:])
        for dc in range(c.DC):
            # x chunk 1 (needed by pair-0 fillers early)
            eng = nc.sync if dc % 2 == 0 else nc.scalar
            eng.dma_start(out=xt2[:, dc, ch:2 * ch], in_=xT[:, dc, ch:2 * ch])
        # interleave the casted-x and bf16-x chunk streams by need time
        for cs in range(ch, c.NT, ch):
            for d2 in range(0, c.DC, 2):
                nc.gpsimd.dma_start(
                    out=x8[:, d2:d2 + 2, cs:cs + ch],
                    in_=xT[:, d2:d2 + 2, cs:cs + ch])
            if cs + ch < c.NT:
                for d2 in range(0, c.DC, 2):
                    nc.gpsimd.dma_start(
                        out=xt2[:, d2:d2 + 2, cs + ch:cs + 2 * ch],
                        in_=xT[:, d2:d2 + 2, cs + ch:cs + 2 * ch])
        for pr in range(c.NPAIR):
            nc.gpsimd.dma_start(
                out=wout_sb[:, pr, :], in_=wout[:, pr, :])

        # ---------------- Phase B: attention (the first q-chunk's
        # v-projection is emitted right after the q/k prologue) ----------
        with (
            tc.tile_pool(name="pt", bufs=c.PT_BUFS) as pt_pool,
            tc.tile_pool(name="otst", bufs=c.OT_BUFS) as ot_pool,
            tc.tile_pool(name="den", bufs=c.DEN_BUFS) as den_pool,
            tc.tile_pool(name="dend", bufs=4, space="DRAM") as dend_pool,
            tc.tile_pool(name="ps_s", bufs=c.PSS_BUFS, space="PSUM") as ps_s,
            tc.tile_pool(name="ps_o", bufs=c.PSO_BUFS, space="PSUM") as ps_o,
            tc.tile_pool(name="ps_f", bufs=2, space="PSUM") as ps_f,
            tc.tile_pool(name="osbB", bufs=3) as outB_pool,
        ):
            def evac_stage1(psO):
                ocps = []
                for h2 in range(2):
                    # evacuate psum to SBUF immediately to free the bank
                    # (GPSIMD cannot read PSUM, so both stay on DVE)
                    ocp = ot_pool.tile([c.VW, c.QC], F32, tag="ocp",
                                       name="ocp", bufs=6)
                    nc.vector.tensor_copy(out=ocp[:], in_=psO[h2][:])
                    ocps.append(ocp)
                return ocps

            def evac_stage2(p, t, ocps):
                qsl_full = slice(t * c.QC, (t + 1) * c.QC)
                # Fold both heads' denominator rows [1, QC] into a [128, *]
                # square via DMA so the DVE reciprocal runs across all 128
                # lanes (~130ns instead of ~3.3us on one lane), then bounce
                # through DRAM for the partition broadcast.  Both hops use
                # identical element orderings, so positions are preserved.
                QW = c.QC // 128
                den_sq = den_pool.tile([128, 2, QW], F32, tag="dsq",
                                       name="den_sq", bufs=3)
                for h2 in range(2):
                    nc.sync.dma_start(
                        out=den_sq[:, h2, :], in_=ocps[h2][c.DH:c.VW, :])
                rec_sq = den_pool.tile([128, 2, QW], F32, tag="rsq",
                                       name="rec_sq", bufs=3)
                nc.vector.reciprocal(out=rec_sq[:], in_=den_sq[:])
                rec_d = dend_pool.tile([2, c.QC], F32, tag="dend",
                                       name="rec_d")
                for h2 in range(2):
                    nc.sync.dma_start(
                        out=rec_d[h2], in_=rec_sq[:, h2, :])
                for h2 in range(2):
                    ocp = ocps[h2]
                    divB = den_pool.tile([c.DH, c.QC], F32, tag="div",
                                         name="divB")
                    nc.sync.dma_start(
                        out=divB[:], in_=rec_d[h2].partition_broadcast(c.DH))
                    if h2 == 0:
                        nc.vector.tensor_tensor(
                            out=OT_sb[p][0:c.DH, qsl_full], in0=ocp[0:c.DH, :],
                            in1=divB[:], op=mybir.AluOpType.mult,
                        )
                    else:
                        ot_st = ot_pool.tile([c.DH, c.QC], BF16, tag="ot",
                                             name="ot_st")
                        nc.gpsimd.tensor_tensor(
                            out=ot_st[:], in0=ocp[0:c.DH, :], in1=divB[:],
                            op=mybir.AluOpType.mult,
                        )
                        nc.sync.dma_start(
                            out=OT_sb[p][c.DH:2 * c.DH, qsl_full],
                            in_=ot_st[:])

            def emit_proj_chunk(nt, ec, split=False):
                esl = slice(ec * c.QC, (ec + 1) * c.QC)
                psP = ps_f.tile([128, c.QC], F32, tag="f", name="psP")
                for pr in range(c.NPAIR):
                    nc.tensor.matmul(
                        psP[:],
                        lhsT=OT_sb[pr][:, nt * 128:(nt + 1) * 128],
                        rhs=wout_sb[:, pr, esl],
                        start=(pr == 0), stop=(pr == c.NPAIR - 1),
                    )
                o_sb = outB_pool.tile([128, c.QC], F32, tag="ob", name="o_sb")
                nc.vector.tensor_copy(out=o_sb[:], in_=psP[:])
                if not split:
                    nc.sync.dma_start(
                        out=out[nt * 128:(nt + 1) * 128, esl], in_=o_sb[:])
                else:
                    # the run's last stores: 4-way partition split across
                    # three trigger engines so the 256KB transfer does not
                    # ride a single 22GB/s DMA engine on the critical tail
                    for i, eng in enumerate(
                            (nc.sync, nc.scalar, nc.gpsimd, nc.sync)):
                        ps = slice(i * 32, (i + 1) * 32)
                        eng.dma_start(
                            out=out[nt * 128 + i * 32:
                                    nt * 128 + (i + 1) * 32, esl],
                            in_=o_sb[ps, :])

            def emit_qk_chunk(pp, ci):
                is_k = ci // c.NQC
                e = (pp, c.NPAIR + pp)[is_k]
                ncc = ci % c.NQC
                nsl = slice(ncc * c.QC, (ncc + 1) * c.QC)
                psqk = ps_f.tile([128, c.QC], F32, tag="f", name="psqk")
                w8 = wk8_sb if is_k else wq8_sb
                # fp8 DoubleRow: two D-chunks per pass, 0.5 cycles/row
                for dcp in range(c.DC // 2):
                    nc.tensor.matmul(
                        psqk[:],
                        lhsT=w8[:, dcp, :, pp * 128:(pp + 1) * 128],
                        rhs=x8[:, 2 * dcp:2 * dcp + 2, nsl],
                        start=(dcp == 0), stop=(dcp == c.DC // 2 - 1),
                        perf_mode=mybir.MatmulPerfMode.DoubleRow,
                    )
                nc.vector.tensor_copy(out=qk_sb[e][:, nsl], in_=psqk[:])

            def emit_v_chunk(nt):
                psv = ps_f.tile([128, c.QF], F32, tag="f", name="psvf")
                for dc in range(c.DC):
                    nc.tensor.matmul(
                        psv[:],
                        lhsT=xt2[:, dc, nt * 128:(nt + 1) * 128],
                        rhs=wv_sb[:, dc, :],
                        start=(dc == 0), stop=(dc == c.DC - 1),
                    )
                nc.vector.tensor_copy(
                    out=V_sb[:, nt, :, 0:c.DH],
                    in_=psv[:].rearrange("p (h f) -> p h f", h=c.HH),
                )

            pending2 = None
            # prologue: the first q-chunk's q/k columns (fp8, early data),
            # then its v rows
            emit_qk_chunk(0, 0)
            emit_qk_chunk(0, c.NQC)
            for nt in range(c.QC // 128):
                emit_v_chunk(nt)
            for p in range(c.NPAIR):
                q_t, k_t = qk_sb[p], qk_sb[c.NPAIR + p]
                for t in range(c.NQC):
                    njt = JPT * t + JPT
                    psO = [ps_o.tile([c.VW, c.QC], F32, tag="o", name=f"psO{_h}")
                           for _h in range(2)]
                    def emit_pv(items):
                        for (h2_, pt_, lo_, j_) in items:
                            nc.tensor.matmul(
                                psO[h2_][:, lo_:],
                                lhsT=V_sb[:, j_, 2 * p + h2_, :],
                                rhs=pt_[:, h2_, lo_:],
                                start=(j_ == 0), stop=(j_ == njt - 1),
                            )

                    pipe = []
                    for j in range(njt):
                        off = j * c.KC - t * c.QC
                        band = off >= 0
                        lo = max(off, 0)
                        jsl = slice(j * c.KC, (j + 1) * c.KC)
                        qsl = slice(t * c.QC + lo, (t + 1) * c.QC)
                        # both heads' scores into one 2-bank psum tile
                        psS = ps_s.tile([128, 2, c.QC], F32, tag="s", name="psS")
                        for h2 in range(2):
                            hsl = slice(h2 * c.DH, (h2 + 1) * c.DH)
                            nc.tensor.matmul(
                                psS[:, h2, lo:], lhsT=k_t[hsl, jsl],
                                rhs=q_t[hsl, qsl], start=True, stop=(not band),
                            )
                        if band:
                            # causal mask add on PE for both heads at once:
                            # psum += I.T @ [mtri | mtri]
                            nc.tensor.matmul(
                                psS[:, :, off:off + c.KC], lhsT=iden_sb[:],
                                rhs=mtri_sb[:], start=False, stop=True,
                                skip_group_check=True,
                            )
                        pt_t = pt_pool.tile([128, 2, c.QC], BF16, tag="pt",
                                            name="pt_t")
                        nc.scalar.activation(
                            out=pt_t[:, :, lo:],
                            in_=psS[:, :, lo:],
                            func=mybir.ActivationFunctionType.Exp,
                            scale=1.0 / 32768.0,
                        )
                        pipe.append([(0, pt_t, lo, j), (1, pt_t, lo, j)])
                        if len(pipe) > 3:
                            emit_pv(pipe.pop(0))
                    # interleave filler work (next pair's qk projection,
                    # or output-projection chunks during the last pair) with
                    # the PV pipe flush so the PE has matmuls to run while
                    # ScalarE finishes the tail exps
                    NTT = c.QC // 128
                    if p == 0:
                        fillers = []
                        if t + 1 < c.NQC:
                            # next q-chunk's V rows and q/k columns
                            fillers += [
                                lambda nt_=nt_: emit_v_chunk(nt_)
                                for nt_ in range((t + 1) * NTT,
                                                 min((t + 2) * NTT, c.NKT))]
                            fillers += [
                                lambda ci=ci: emit_qk_chunk(0, ci)
                                for ci in (t + 1, c.NQC + t + 1)]
                        fillers += [lambda i=i: emit_qk_chunk(1, 2 * t + i)
                                    for i in range(2)]
                    elif p + 1 < c.NPAIR:
                        fillers = [lambda i=i: emit_qk_chunk(p + 1, 2 * t + i)
                                   for i in range(2)]
                    elif t >= 2:
                        # two-chunk lag keeps the filler projections clear
                        # of the still-in-flight evac chain of chunk t-1
                        tp = t - 2
                        fillers = [
                            lambda nt_=nt_, ec_=ec_: emit_proj_chunk(nt_, ec_)
                            for nt_ in range(tp * NTT, (tp + 1) * NTT)
                            for ec_ in range(c.E // c.QC)]
                    else:
                        fillers = []
                    while fillers or pipe:
                        if fillers:
                            fillers.pop(0)()
                        if pipe:
                            emit_pv(pipe.pop(0))
                    ocps = evac_stage1(psO)
                    if pending2 is not None:
                        evac_stage2(*pending2)
                        pending2 = None
                    if p == c.NPAIR - 1:
                        # last pair: evacuate immediately so the output
                        # projection can chase with only one chunk of lag
                        evac_stage2(p, t, ocps)
                    else:
                        pending2 = (p, t, ocps)

            if pending2 is not None:
                evac_stage2(*pending2)

            # final q-chunks' output projection, still inside the same pool
            # scope (no drain barrier before it); chunk NQC-2 is ready
            # immediately and overlaps the last chunk's evac chain
            NTTf = c.QC // 128
            for nt in range((c.NQC - 2) * NTTf, c.NT // 128):
                for ec in range(c.E // c.QC):
                    emit_proj_chunk(nt, ec, split=(nt >= c.NT // 128 - 2))

    nc.compile()
    return nc


# ---------------------------------------------------------------- host side
def make_core_inputs(xb, w_qkv, w_out, mask, cfg, half):
    c = cfg
    D = c.D
    K8_SCALE = 64.0  # fp8 weight scale; 1/(64*64*sqrt(DH)) folds into exp
    heads = range(half * c.HH, (half + 1) * c.HH)
    q_rows = np.concatenate(
        [w_qkv[h * c.DH:(h + 1) * c.DH, :] for h in heads])
    k_rows = np.concatenate(
        [w_qkv[D + h * c.DH:D + (h + 1) * c.DH, :] for h in heads])
    v_rows = np.concatenate(
        [w_qkv[2 * D + h * c.DH:2 * D + (h + 1) * c.DH, :] for h in heads])

    def pack8(rows):
        # [p, dcp, o, f] = 64*w[f, (2*dcp+o)*128 + p] in fp8e4m3
        return np.ascontiguousarray(
            (rows.T * K8_SCALE).reshape(c.DC // 2, 2, 128, c.QF)
            .transpose(2, 0, 1, 3)).astype(ml_dtypes.float8_e4m3fn)

    wq8_t = pack8(q_rows)
    wk8_t = pack8(k_rows)
    wv_t = np.ascontiguousarray(
        v_rows.T.reshape(c.DC, 128, c.QF).transpose(1, 0, 2)).astype(
            ml_dtypes.bfloat16)
    wo = w_out[:, half * c.QF:(half + 1) * c.QF].T  # [QF, E]
    wo = np.ascontiguousarray(
        wo.reshape(c.NPAIR, 128, c.E).transpose(1, 0, 2)).astype(
            ml_dtypes.bfloat16)
    mt = np.where(mask[0, 0, :c.KC, :c.KC].T != 0, 0.0,
                  c.MASK_NEG).astype(ml_dtypes.bfloat16)
    mt2 = np.ascontiguousarray(
        np.repeat(mt[:, None, :], 2, axis=1))  # [128, 2, KC]
    xtr = np.ascontiguousarray(
        xb.T.reshape(c.DC, 128, c.NT).transpose(1, 0, 2)).astype(
            ml_dtypes.bfloat16)
    return {
        "xT": xtr,
        "wq8": wq8_t,
        "wk8": wk8_t,
        "wv": wv_t,
        "wout": wo,
        "mtri": mt2,
        "iden": np.eye(128, dtype=ml_dtypes.bfloat16),
    }


_CACHE = {}


def run_sharded(x, mask, w_qkv, w_out, trace=False, trace_cores=None):
    """Shard inputs over 8 cores, run the bass kernel, gather full output.
    Returns (out [B,N,D] f32, BassKernelResults)."""
    # the axon PJRT backend is required for execution; guard against a
    # caller environment that overrode JAX_PLATFORMS before jax init
    if 'jax' not in sys.modules and 'axon' not in os.environ.get(
            'JAX_PLATFORMS', 'axon'):
        os.environ['JAX_PLATFORMS'] = 'axon'
    from concourse.bass_utils import run_bass_kernel_spmd

    cfg = Cfg()
    B = x.shape[0]
    n_cores = 2 * B
    if 'nc' not in _CACHE:
        _CACHE['nc'] = build(cfg)
    nc = _CACHE['nc']

    x = np.asarray(x, np.float32)
    mask = np.asarray(mask)
    w_qkv = np.asarray(w_qkv, np.float32)
    w_out = np.asarray(w_out, np.float32)

    in_maps = []
    for core in range(n_cores):
        b, half = core // 2, core % 2
        in_maps.append(make_core_inputs(x[b], w_qkv, w_out, mask, cfg, half))

    if trace:
        _install_ntff_hook()
    res = run_bass_kernel_spmd(
        nc, in_maps, core_ids=list(range(n_cores)), trace=trace,
        trace_cores=trace_cores)
    outs = []
    for b in range(B):
        outs.append(res.results[2 * b]["out"].astype(np.float64)
                    + res.results[2 * b + 1]["out"].astype(np.float64))
    return np.stack(outs).astype(np.float32), res


def kernel(x, mask, w_qkv, w_out):
    out, _ = run_sharded(x, mask, w_qkv, w_out, trace=False)
    return out

